# revision 33
# baseline (speedup 1.0000x reference)
"""Multi-head attention (B=2, S=2048, D=1024, H=16) on 8 NeuronCores.

Sharding: core c handles batch b = c//4 and head-group g = c%4 (4 heads,
F = 256 features). Data-parallel over B, tensor-parallel over heads:
Wq/Wk/Wv column-sliced, Wo row-sliced; host sums the 8 partial outputs.

v2 changes vs baseline:
  - fp16 everywhere (x, weights, qT/kT, v2, keep, e, ctxT, wo): halves DMA,
    enables FWL weight loads on the PE.
  - sq-outer / hp-inner phase-2 loop; each keep tile is loaded once.
  - U-matmuls skewed one sk-iteration behind the score matmuls so the
    in-order PE queue never waits on the EXP->mask-mul chain.
  - per-sq finalize (denominators + ctx scale + output projection + DMA out)
    issued one sq-block late so it pipelines under the next block's compute.
"""

import numpy as np

import concourse.tile as tile
from concourse import bacc, mybir
from concourse.bass_utils import run_bass_kernel_spmd

B, S, D, H = 2, 2048, 1024, 16
DH = D // H  # 64
NCORES = 8
GROUPS = 4  # head groups (cores per batch)
HL = H // GROUPS  # 4 heads per core
F = HL * DH  # 256 local features
SQC = 512  # sq chunk width
NSQ = S // SQC  # 4
SKT = S // 128  # 16 sk tiles
PD = D // 128  # 8 contraction chunks
CW = 512  # phase-1 s-chunk width
NPC = S // CW

FP32 = mybir.dt.float32
FP16 = mybir.dt.float16

_CACHE = {}


def _build():
    nc = bacc.Bacc("TRN2", target_bir_lowering=False, debug=False)

    xq_d = nc.dram_tensor("xqT", [NPC, 128, PD, CW], FP16, kind="ExternalInput").ap()
    xk_d = nc.dram_tensor("xkT", [NPC, 128, PD, CW], FP16, kind="ExternalInput").ap()
    xv_d = nc.dram_tensor("xvT", [NPC, 128, PD, CW], FP16, kind="ExternalInput").ap()
    keep_d = nc.dram_tensor("keepT", [NSQ, 128, SKT, SQC], FP16, kind="ExternalInput").ap()
    wq_d = nc.dram_tensor("Wq", [128, PD, F], FP16, kind="ExternalInput").ap()
    wk_d = nc.dram_tensor("Wk", [128, PD, F], FP16, kind="ExternalInput").ap()
    wv_d = nc.dram_tensor("Wv", [128, PD, F], FP16, kind="ExternalInput").ap()
    wo_d = nc.dram_tensor("Wo", [128, 2, D], FP16, kind="ExternalInput").ap()
    bq_d = nc.dram_tensor("bq", [F, 1], FP32, kind="ExternalInput").ap()
    bk_d = nc.dram_tensor("bk", [F, 1], FP32, kind="ExternalInput").ap()
    bv_d = nc.dram_tensor("bv", [1, F], FP32, kind="ExternalInput").ap()
    out_d = nc.dram_tensor("out", [S, D], FP16, kind="ExternalOutput").ap()

    Exp = mybir.ActivationFunctionType.Exp
    Ln = mybir.ActivationFunctionType.Ln

    with tile.TileContext(nc) as tc:
        with tc.tile_pool(name="persist", bufs=1) as pp, \
             tc.tile_pool(name="keepp", bufs=1) as keepp, \
             tc.tile_pool(name="p2e", bufs=1) as p2e, \
             tc.tile_pool(name="finp", bufs=1) as finp:
            qT = pp.tile([128, 2, S], FP16, tag="qT")  # 2 f-chunks (=head pairs)
            kT = pp.tile([128, 2, S], FP16, tag="kT")
            v2 = pp.tile([128, SKT, HL, DH + 1], FP16, tag="v2")
            ctxT = pp.tile([128, 2, S], FP16, tag="ctxT")
            wo = pp.tile([128, 2, D], FP16, tag="wo")
            bq_sb = pp.tile([128, 2, 1], FP32, tag="bq")
            bk_sb = pp.tile([128, 2, 1], FP32, tag="bk")
            bv_bc = pp.tile([128, F], FP32, tag="bvbc")
            wu = pp.tile([128, 64], FP16, tag="wu")
            bv_row = pp.tile([1, F], FP32, tag="bvrow")

            def load_small():
                # biases: many tiny descriptors — keep them behind the weights
                nc.scalar.dma_start(out=bv_row[:], in_=bv_d)
                nc.scalar.dma_start(out=bq_sb[:],
                                    in_=bq_d.rearrange("(c p) o -> p c o", p=128))
                nc.scalar.dma_start(out=bk_sb[:],
                                    in_=bk_d.rearrange("(c p) o -> p c o", p=128))
                nc.scalar.dma_start(out=wo[:], in_=wo_d)
            nc.vector.memset(wu[:], 0.125)
            nc.vector.memset(v2[:, :, :, DH:DH + 1], 1.0)

            def load_keep(sq, eng):
                t = keepp.tile([128, SKT, SQC], FP16, tag="keep",
                               name=f"keep_{sq}", bufs=2)
                eng.dma_start(out=t[:], in_=keep_d[sq])
                return t

            keep_tiles = {}

            # ---- phase 1: projections ----
            with tc.tile_pool(name="p1", bufs=2) as p1, \
                 tc.tile_pool(name="p1w", bufs=1) as p1w:
                # weights on the scalar queue (its compute comes later), x
                # tiles on sync/gpsimd which carry no compute — so tile
                # prefetch is never serialized behind PSUM-dependent ops.
                wq = p1w.tile([128, PD, F], FP16, tag="wq")
                wk = p1w.tile([128, PD, F], FP16, tag="wk")
                wv = p1w.tile([128, PD, F], FP16, tag="wv")
                nc.scalar.dma_start(out=wk[:], in_=wk_d)
                nc.scalar.dma_start(out=wq[:], in_=wq_d)
                nc.gpsimd.dma_start(out=wv[:], in_=wv_d)
                load_small()

                with tc.tile_pool(name="psum1", bufs=2, space="PSUM") as ps1:
                    # warm the PE while the first x tiles stream in
                    wm_ps = ps1.tile([128, 512], FP32, tag="q_ps", name="warm")
                    for _ in range(40):
                        nc.tensor.matmul(wm_ps[0:64, 0:64], wu[:], wu[:],
                                         start=True, stop=True)
                    for s4 in range(NPC):
                        sl = slice(s4 * CW, (s4 + 1) * CW)
                        xq_sl = p1.tile([128, PD, CW], FP16, tag="xq")
                        xk_sl = p1.tile([128, PD, CW], FP16, tag="xk")
                        xv_sl = p1.tile([128, PD, CW], FP16, tag="xv")
                        nc.sync.dma_start(out=xk_sl[:], in_=xk_d[s4])
                        nc.sync.dma_start(out=xq_sl[:], in_=xq_d[s4])
                        nc.gpsimd.dma_start(out=xv_sl[:], in_=xv_d[s4])
                        if s4 == 0:
                            # after xv0 so it doesn't block the x stream
                            nc.gpsimd.partition_broadcast(bv_bc[:], bv_row[:])
                        if s4 == NPC - 1:
                            keep_tiles[0] = load_keep(0, nc.gpsimd)

                        for fc in range(2):
                            fsl = slice(fc * 128, (fc + 1) * 128)
                            k_ps = ps1.tile([128, CW], FP32, tag="k_ps")
                            for d in range(PD):
                                nc.tensor.matmul(k_ps[:], wk[:, d, fsl], xk_sl[:, d, :],
                                                 start=(d == 0), stop=(d == PD - 1))
                            nc.scalar.add(kT[:, fc, sl], k_ps[:], bk_sb[:, fc, :])

                            q_ps = ps1.tile([128, CW], FP32, tag="q_ps")
                            for d in range(PD):
                                nc.tensor.matmul(q_ps[:], wq[:, d, fsl], xq_sl[:, d, :],
                                                 start=(d == 0), stop=(d == PD - 1))
                            nc.scalar.add(qT[:, fc, sl], q_ps[:], bq_sb[:, fc, :])

                        for m in range(CW // 128):  # s-subtiles of 128
                            ti = s4 * (CW // 128) + m
                            msl = slice(m * 128, (m + 1) * 128)
                            v_ps = ps1.tile([128, F], FP32, tag="v_ps")
                            for d in range(PD):
                                nc.tensor.matmul(v_ps[:], xv_sl[:, d, msl], wv[:, d, :],
                                                 start=(d == 0), stop=(d == PD - 1))
                            nc.vector.tensor_add(
                                v2[:, ti, :, 0:DH],
                                v_ps[:].rearrange("p (h d) -> p h d", h=HL),
                                bv_bc[:].rearrange("p (h d) -> p h d", h=HL),
                            )

            # ---- phase 2+3: attention, sq outer ----
            with tc.tile_pool(name="psum_st", bufs=1, space="PSUM") as ps_st, \
                 tc.tile_pool(name="psum_u", bufs=1, space="PSUM") as ps_u, \
                 tc.tile_pool(name="psum_o", bufs=1, space="PSUM") as ps_o:

                def scale_hp(sq, hp, r4):
                    # sums live at partitions {0,32,64,96}; reciprocal cost is
                    # free-dim driven so running all 128 partitions is free and
                    # the junk rows in between are never read.
                    qsl = slice(sq * SQC, (sq + 1) * SQC)
                    for j in range(2):
                        j4 = 2 * hp + j
                        rt = finp.tile([1, SQC], FP16, tag="rtmp", bufs=2,
                                       name=f"rt_{sq}_{j4}")
                        nc.vector.tensor_copy(rt[:], r4[32 * j4:32 * j4 + 1, :])
                        rb = finp.tile([128, SQC], FP16, tag="rb", bufs=2,
                                       name=f"rb_{sq}_{j4}")
                        nc.gpsimd.partition_broadcast(rb[:], rt[:])
                        nc.vector.tensor_mul(
                            ctxT[j * DH:(j + 1) * DH, hp, qsl],
                            ctxT[j * DH:(j + 1) * DH, hp, qsl],
                            rb[j * DH:(j + 1) * DH, :])

                def make_p3(sq):
                    def run():
                        for ti4 in range(4):
                            ti = sq * 4 + ti4
                            tsl = slice(ti * 128, (ti + 1) * 128)
                            for n in range(2):
                                nsl = slice(n * 512, (n + 1) * 512)
                                o_ps = ps_o.tile([128, 512], FP32, tag="o",
                                                 bufs=2, name=f"o_{sq}_{ti4}_{n}")
                                for fc in range(2):
                                    nc.tensor.matmul(o_ps[:], ctxT[:, fc, tsl],
                                                     wo[:, fc, nsl],
                                                     start=(fc == 0), stop=(fc == 1))
                                o_sb = finp.tile([128, 512], FP16, tag="osb",
                                                 bufs=4, name=f"osb_{sq}_{ti4}_{n}")
                                nc.vector.tensor_copy(o_sb[:], o_ps[:])
                                nc.sync.dma_start(out=out_d[tsl, nsl], in_=o_sb[:])
                    return run

                pending = None
                for sq in range(NSQ):
                    if sq + 1 < NSQ:
                        keep_tiles[sq + 1] = load_keep(sq + 1, nc.sync)
                    keep = keep_tiles.pop(sq)
                    qsl = slice(sq * SQC, (sq + 1) * SQC)
                    sums_sb = finp.tile([128, SQC], FP32, tag="sums", bufs=2,
                                        name=f"sums_{sq}")
                    r4 = finp.tile([128, SQC], FP32, tag="r4", bufs=2,
                                   name=f"r4_{sq}")
                    for hp in range(2):
                        if hp == 1 and pending is not None:
                            # run the previous block's finalize under hp1's
                            # compute so nothing is left for the sq boundary
                            pending()
                            pending = None
                        u = [ps_u.tile([128, 1, SQC], FP32, tag="u", bufs=2,
                                       name=f"u_{sq}_{hp}_{j}")
                             for j in range(2)]
                        pend = []  # U-matmuls run SKEW iterations behind

                        def issue_u(pe2, psk):
                            for j in range(2):
                                nc.tensor.matmul(
                                    u[j][0:DH + 1, 0, :],
                                    v2[:, psk, 2 * hp + j, :],
                                    pe2[:, j, :],
                                    start=(psk == 0), stop=(psk == SKT - 1),
                                )

                        SKEW = 3
                        for sk in range(SKT):
                            ksl = slice(sk * 128, (sk + 1) * 128)
                            st_ps = ps_st.tile([128, 2, SQC], FP32, tag="st",
                                               bufs=2, name=f"st_{sq}_{hp}_{sk}")
                            nc.tensor.matmul(st_ps[:, 0, :], kT[0:64, hp, ksl],
                                             qT[0:64, hp, qsl], start=True, stop=True,
                                             tile_position=(0, 0))
                            nc.tensor.matmul(st_ps[:, 1, :], kT[64:128, hp, ksl],
                                             qT[64:128, hp, qsl], start=True, stop=True,
                                             tile_position=(64, 0))
                            if len(pend) >= SKEW:
                                issue_u(*pend.pop(0))
                            e_sb = p2e.tile([128, 2, SQC], FP16, tag="e", bufs=3)
                            nc.scalar.activation(e_sb[:], st_ps[:], Exp, scale=0.125)
                            e2 = p2e.tile([128, 2, SQC], FP16, tag="e2",
                                          bufs=SKEW + 2)
                            nc.vector.tensor_mul(e2[:, 0, :], e_sb[:, 0, :],
                                                 keep[:, sk, :])
                            nc.vector.tensor_mul(e2[:, 1, :], e_sb[:, 1, :],
                                                 keep[:, sk, :])
                            pend.append((e2, sk))
                        for p in pend:
                            issue_u(*p)
                        for j in range(2):
                            j4 = 2 * hp + j
                            nc.vector.tensor_copy(sums_sb[32 * j4:32 * j4 + 1, :],
                                                  u[j][DH:DH + 1, 0, :])
                            nc.vector.tensor_copy(ctxT[j * DH:(j + 1) * DH, hp, qsl],
                                                  u[j][0:DH, 0, :])
                        if sq == NSQ - 1:
                            # last block: scale inline so the tail is short
                            nc.vector.reciprocal_approx_fast(r4[:], sums_sb[:])
                            scale_hp(sq, hp, r4)
                    if sq < NSQ - 1:
                        def make_fin(sq, sums_sb, r4):
                            def run():
                                nc.vector.reciprocal_approx_fast(r4[:], sums_sb[:])
                                scale_hp(sq, 0, r4)
                                scale_hp(sq, 1, r4)
                                make_p3(sq)()
                            return run
                        pending = make_fin(sq, sums_sb, r4)
                    else:
                        pending = make_p3(sq)
                pending()

    nc.compile()
    return nc


def _tile_x(xT):
    # (D, S) -> (NPC, 128, PD, CW); [s4, p, c, j] = xT[c*128+p, s4*CW+j]
    return np.ascontiguousarray(
        xT.reshape(PD, 128, NPC, CW).transpose(2, 1, 0, 3))


def kernel(query, key, value, mask, Wq, bq, Wk, bk, Wv, bv, Wo, bo, **_):
    if "nc" not in _CACHE:
        _CACHE["nc"] = _build()
    nc = _CACHE["nc"]

    query = np.asarray(query, np.float16)
    key = np.asarray(key, np.float16)
    value = np.asarray(value, np.float16)
    mask = np.asarray(mask)
    Wq = np.asarray(Wq, np.float16)
    Wk = np.asarray(Wk, np.float16)
    Wv = np.asarray(Wv, np.float16)
    Wo = np.asarray(Wo, np.float16)
    bq = np.asarray(bq, np.float32)
    bk = np.asarray(bk, np.float32)
    bv = np.asarray(bv, np.float32)
    bo = np.asarray(bo, np.float32)

    xT = {}
    keepT = {}
    for b in range(B):
        xT[b] = (
            _tile_x(query[b].T),
            _tile_x(key[b].T),
            _tile_x(value[b].T),
        )
        kp = (~mask[b]).T.astype(np.float16)  # (sk, sq)
        keepT[b] = np.ascontiguousarray(
            kp.reshape(SKT, 128, NSQ, SQC).transpose(2, 1, 0, 3))

    wsl = {}
    for g in range(GROUPS):
        fs = slice(g * F, (g + 1) * F)
        wsl[g] = (
            np.ascontiguousarray(Wq[:, fs].reshape(PD, 128, F).transpose(1, 0, 2)),
            np.ascontiguousarray(Wk[:, fs].reshape(PD, 128, F).transpose(1, 0, 2)),
            np.ascontiguousarray(Wv[:, fs].reshape(PD, 128, F).transpose(1, 0, 2)),
            np.ascontiguousarray(
                Wo[fs, :].reshape(2, 128, D).transpose(1, 0, 2)),
            np.ascontiguousarray(bq[fs].reshape(F, 1)),
            np.ascontiguousarray(bk[fs].reshape(F, 1)),
            np.ascontiguousarray(bv[fs].reshape(1, F)),
        )

    in_maps = []
    for c in range(NCORES):
        b, g = c // GROUPS, c % GROUPS
        wq_s, wk_s, wv_s, wo_s, bq_s, bk_s, bv_s = wsl[g]
        in_maps.append({
            "xqT": xT[b][0], "xkT": xT[b][1], "xvT": xT[b][2],
            "keepT": keepT[b],
            "Wq": wq_s, "Wk": wk_s, "Wv": wv_s, "Wo": wo_s,
            "bq": bq_s, "bk": bk_s, "bv": bv_s,
        })

    res = run_bass_kernel_spmd(nc, in_maps, core_ids=list(range(NCORES)))
    outs = [r["out"] for r in res.results]
    full = np.empty((B, S, D), np.float32)
    for b in range(B):
        acc = outs[GROUPS * b].astype(np.float32)
        for g in range(1, GROUPS):
            acc = acc + outs[GROUPS * b + g]
        full[b] = acc + bo
    return full
